# revision 12
# baseline (speedup 1.0000x reference)
"""DLRM dot-interaction kernel for Trainium2 (8 NeuronCores, batch-sharded).

Per sample b: T = concat(dense[b], embs[b]) -> [27, 128]; Z = T @ T^T;
output = strict upper triangle of Z -> [351] fp32.

Per-core plan (2048 samples, 16 blocks of 128):
  - SWDGE cast-DMA loads block groups as X [128 b, (t,f,d)] fp16
    (gpsimd carries ONLY input loads so descriptor gen is never stalled).
  - PE stream is software-pipelined per block: transposes(t) are emitted
    before Gram(t-1) so the PSUM->SBUF copy latency of Tt(t) hides under
    the previous block's matmuls and the PE never idles (HAM stays warm).
  - Transposes: 27 fp16 PE transposes per block -> fp16 PSUM -> DVE copy
    to f-major Tt [128 d, f*128+b] (pads f>=27 left as garbage; never read).
  - Gram: per-sample fp16 matmul, lhsT = Ttr[:, b, :27], moving 32 cols,
    out -> PSUM [27, 32] at col-tile 32*(b%4); DVE copies PSUM -> Zs
    [(g,m) part, (q,n)] fp16.
  - Bounce: scalar-ring HWDGE write per block to m-major DRAM scratch
    (row m*4+g, 2KB runs); sync-ring HWDGE gather back sample-major into
    Zb [(g,q) part, (t,m,n)] (64B runs). Writes and gathers sit on
    different NX queues so their completion waits overlap across blocks.
  - Pack: 26 ACT copies per quarter (fp16 -> fp32) -> Pk, then 4 HWDGE
    out DMAs, both deferred one quarter so their gather-waits never block
    the next quarter's scratch writes on the scalar queue.
"""

import numpy as np

B, NUM_EMBS, D = 16384, 26, 128
N_CORES = 8
BC = B // N_CORES  # 2048 samples per core
BLK = 128          # samples per block
NF = NUM_EMBS + 1  # 27 features
FP = 32            # feature pitch (27 + 5 pad)
NPAIR = NF * (NF - 1) // 2  # 351

_CACHE = {}


def build(bc=BC):
    import concourse.bacc as bacc
    import concourse.mybir as mybir
    from concourse.tile import TileContext
    from concourse.masks import make_identity

    fp16 = mybir.dt.float16
    fp32 = mybir.dt.float32

    nc = bacc.Bacc("TRN2", target_bir_lowering=False, debug=False)
    dense_t = nc.dram_tensor("dense", (bc, D), fp32, kind="ExternalInput")
    embs_t = nc.dram_tensor("embs", (bc, NUM_EMBS, D), fp32, kind="ExternalInput")
    out_t = nc.dram_tensor("out", (bc, NPAIR), fp32, kind="ExternalOutput")

    nblk = bc // BLK
    QBLK = 4  # blocks per quarter (pack/out granularity)
    nq = nblk // QBLK

    groups = []
    b = 0
    head = [1, 1, 2]
    while b < nblk:
        sz = min(head.pop(0) if head else 4, nblk - b)
        groups.append((b, sz))
        b += sz
    g_of = {}
    for gs, sz in groups:
        for i in range(sz):
            g_of[gs + i] = (gs, sz)

    with TileContext(nc) as tc:
        with (
            tc.tile_pool(name="consts", bufs=1) as consts,
            tc.tile_pool(name="xin", bufs=3) as xpool,
            tc.tile_pool(name="tt", bufs=3) as ttpool,
            tc.tile_pool(name="zsb", bufs=6) as zpool,
            tc.tile_pool(name="zb", bufs=2) as zbpool,
            tc.tile_pool(name="pk", bufs=2) as pkpool,
            tc.tile_pool(name="tp", bufs=4, space="PSUM") as tppool,
            tc.tile_pool(name="zp", bufs=4, space="PSUM") as zppool,
            tc.tile_pool(name="dscr", bufs=6, space="DRAM") as dpool,
        ):
            ident = consts.tile([128, 128], fp16)
            make_identity(nc, ident)

            dview = dense_t.ap()
            eview = embs_t.ap().rearrange("b f d -> b (f d)")
            oview = out_t.ap()

            X = None
            xof = {}      # blk -> (X tile, col offset)
            tts = {}      # blk -> Tt tile
            zbs = {}      # qtr -> Zb tile
            pend_pack = []  # [(qtr, Zb)] quarters whose pack+out are deferred

            def load_inputs(blk):
                nonlocal X
                gs, gsz = g_of[blk]
                if blk == gs:
                    X = xpool.tile([BLK, gsz * NF * D], fp16, tag="X")
                    dsrc = dview[gs * BLK : (gs + gsz) * BLK].rearrange(
                        "(t b) d -> b t d", t=gsz
                    )
                    xd = X.rearrange("b (t c) -> b t c", t=gsz)
                    nc.gpsimd.dma_start(out=xd[:, :, 0:D], in_=dsrc)
                    esrc = eview[gs * BLK : (gs + gsz) * BLK].rearrange(
                        "(t b) c -> b t c", t=gsz
                    )
                    nc.gpsimd.dma_start(out=xd[:, :, D:], in_=esrc)
                xof[blk] = (X, (blk - gs) * NF * D)

            def transposes(blk):
                Xt, xoff = xof[blk]
                Tt = ttpool.tile([128, FP * D], fp16, tag="Tt")
                for c0, cf in ((0, 8), (8, 8), (16, 8), (24, 3)):
                    tp = tppool.tile([128, 8 * BLK], fp16, tag="tp")
                    for j in range(cf):
                        f = c0 + j
                        nc.tensor.transpose(
                            tp[:, j * BLK : (j + 1) * BLK],
                            Xt[:, xoff + f * D : xoff + (f + 1) * D],
                            ident,
                        )
                    nc.vector.tensor_copy(
                        out=Tt[:, c0 * BLK : (c0 + cf) * BLK],
                        in_=tp[:, : cf * BLK],
                    )
                tts[blk] = Tt

            def gram_and_bounce(blk):
                qtr, t = blk // QBLK, blk % QBLK
                if t == 0:
                    Zb = zbpool.tile([128, QBLK * NF * FP], fp16, tag="Zb", name=f"Zb{qtr}")
                    zbs[qtr] = Zb
                Tt = tts.pop(blk)
                Ttr = Tt.rearrange("d (f b) -> d b f", b=BLK)
                Zs = zpool.tile([128, 32 * FP], fp16, tag="Zs")
                for qt in range(0, 32, 16):
                    zp = zppool.tile([128, 16 * FP], fp32, tag="zp")
                    for q in range(16):
                        for g in range(4):
                            bloc = (qt + q) * 4 + g
                            nc.tensor.matmul(
                                zp[32 * g : 32 * g + NF, q * FP : (q + 1) * FP],
                                Ttr[:, bloc, :NF],
                                Ttr[:, bloc, :],
                                start=True,
                                stop=True,
                                tile_position=(0, 32 * g),
                            )
                    nc.vector.tensor_copy(
                        out=Zs[:, qt * FP : (qt + 16) * FP],
                        in_=zp[:, : 16 * FP],
                    )
                # bounce: write (scalar ring) -> gather (sync ring)
                scr = dpool.tile([128, 32 * FP], fp16, tag="scr")
                wdst = scr.rearrange("(m g) c -> g m c", g=4)
                nc.scalar.dma_start(out=wdst, in_=Zs[:, :])
                rsrc = scr.rearrange("(m a) (b n) -> (a b) m n", a=4, n=FP)
                zbt = zbs[qtr][:, t * NF * FP : (t + 1) * NF * FP].rearrange(
                    "p (m n) -> p m n", n=FP
                )
                nc.sync.dma_start(out=zbt, in_=rsrc[:, :NF, :])

            def pack_and_out(qtr, Zb):
                Pk = pkpool.tile([128, QBLK * NPAIR], fp32, tag="Pk")
                zbp = Zb.rearrange("p (t c) -> p t c", t=QBLK)
                pkp = Pk.rearrange("p (t c) -> p t c", t=QBLK)
                off = 0
                for m in range(NF - 1):
                    ln = NF - 1 - m
                    nc.scalar.copy(
                        pkp[:, :, off : off + ln],
                        zbp[:, :, m * FP + m + 1 : m * FP + NF],
                    )
                    off += ln
                b0q = qtr * QBLK * BLK
                ovq = oview[b0q : b0q + QBLK * BLK].rearrange(
                    "(t q g) p -> g q t p", g=4, t=QBLK
                )
                pk4 = Pk.rearrange("(g q) (t c) -> g q t c", g=4, t=QBLK)
                for g in range(4):
                    nc.scalar.dma_start(out=ovq[g], in_=pk4[g])

            # ---- software-pipelined emission ----
            load_inputs(0)
            transposes(0)
            for blk in range(1, nblk):
                load_inputs(blk)
                transposes(blk)
                gram_and_bounce(blk - 1)
                qtr, t = (blk - 1) // QBLK, (blk - 1) % QBLK
                if t == QBLK - 1:
                    pend_pack.append((qtr, zbs.pop(qtr)))
                # defer pack+out until 2 blocks into the next quarter so its
                # gather-waits never stall the scalar queue's scratch writes
                if pend_pack and blk % QBLK == 2:
                    pack_and_out(*pend_pack.pop(0))
            gram_and_bounce(nblk - 1)
            pend_pack.append((nq - 1, zbs.pop(nq - 1)))
            for item in pend_pack:
                pack_and_out(*item)

    nc.compile()
    return nc


def _get(bc=BC):
    if bc not in _CACHE:
        _CACHE[bc] = build(bc)
    return _CACHE[bc]


def kernel(dense: np.ndarray, embs: np.ndarray) -> np.ndarray:
    from concourse import bass_utils

    dense = np.ascontiguousarray(np.asarray(dense, dtype=np.float32))
    embs = np.ascontiguousarray(np.asarray(embs, dtype=np.float32))
    assert dense.shape == (B, D) and embs.shape == (B, NUM_EMBS, D)

    nc = _get()
    dsh = dense.reshape(N_CORES, BC, D)
    esh = embs.reshape(N_CORES, BC, NUM_EMBS, D)
    in_maps = [{"dense": dsh[i], "embs": esh[i]} for i in range(N_CORES)]
    res = bass_utils.run_bass_kernel_spmd(nc, in_maps, core_ids=list(range(N_CORES)))
    return np.concatenate([r["out"] for r in res.results], axis=0)
